# revision 38
# baseline (speedup 1.0000x reference)
"""Trainium2 Bass kernel for GNN message passing (gather + segment_sum).

out[i] = sum_{e: dst[e]==i} x[src[e]]   with x [100000, 64] f32,
edge_index [2, 1600000] int64.

Strategy (8 NeuronCores, SPMD), v2:
  - Destination nodes sharded across cores: core c owns dst rows
    [c*12500, (c+1)*12500), padded to a 12544-row output slab whose row
    order makes every device write contiguous (host un-permutes).
  - Source nodes split into 4 chunks of 25000 rows so dma_gather's int16
    indices stay in range. Each chunk region in HBM carries the 25000 x
    rows, a zero pad row, and per-class scratch rows.
  - Per (node, chunk): W=3 level-1 slots. Degree d<=3 fits entirely; for
    d>3 slots 0-1 hold ranks 0-1 and slot 2 is a POINTER to a scratch
    row holding the partial sum of ranks 2.. computed by an overflow
    class gather (classes sized by overflow count: P in {2,3,4,6,8,16}).
    No separate patch phase: the pointer rides the level-1 gather.
  - Device: overflow classes first (gather -> small tree reduce ->
    scratch write), then 14 level-1 tiles, each one 4-chunk mega gather
    into a single staging tile reduced with 4 large strided vector ops.
    SWDGE queue q handles chunk q throughout -> 4-way parallel
    descriptor generation.
"""

import sys

if "/opt/trn_rl_repo" not in sys.path:
    sys.path.insert(0, "/opt/trn_rl_repo")

import numpy as np

N = 100000
D = 64
N_CORES = 8
ROWS_PER_CORE = N // N_CORES            # 12500
NODE_TILE = 896                         # 7 groups of 128 nodes
GROUPS_PER_TILE = NODE_TILE // 128      # 7
N_TILES = 14
ROWS_PAD = NODE_TILE * N_TILES          # 12544
N_CHUNKS = 4
CHUNK = N // N_CHUNKS                   # 25000
PAD_IDX = CHUNK                         # gather index of the zero row
W = 3                                   # level-1 slots per (node, chunk)
TILE_SLOTS = NODE_TILE * W              # 2688 level-1 indices per (tile, chunk)
IDX_COLS = TILE_SLOTS // 16             # 168

# overflow classes: (lo_deg, hi_deg, P slots). Node with deg d>W gets
# ranks 0..W-2 in level 1 + pointer; class holds d-(W-1) ranks, so
# require hi - (W-1) <= P.
CLASSES = ((4, 4, 2), (5, 5, 3), (6, 6, 4), (7, 8, 6), (9, 10, 8), (11, 18, 16))
# subblock group counts: cap staging at ~1536 f32 cols per partition
CLS_SUBG = {2: 8, 3: 5, 4: 4, 6: 2, 8: 2, 16: 1}

_PROG_CACHE = {}


def _wrap16(a):
    """[..., L] int -> [..., 128, L/16] int16 in the dma_gather index layout:
    position i at [i % 16, i // 16], replicated to all queue core pairs."""
    a = np.ascontiguousarray(a.astype(np.int16))
    L = a.shape[-1]
    assert L % 16 == 0
    t = a.reshape(a.shape[:-1] + (L // 16, 16))
    t = np.swapaxes(t, -1, -2)
    reps = (1,) * (a.ndim - 1) + (8, 1)
    return np.ascontiguousarray(np.tile(t, reps))


DISPATCH_TILE = 896  # node-tiles written per level-1 DMA dispatch


def _slab_row(n):
    """Node index within a core -> output slab row (contiguous tile DMAs)."""
    t = n // DISPATCH_TILE
    w = n % DISPATCH_TILE
    g = w // 128
    r = w % 128
    return t * DISPATCH_TILE + r * (DISPATCH_TILE // 128) + g


def _gather_order(A, P):
    """[..., nodes(G*128), P] slots -> flat gather list order (g, k, r)."""
    G = A.shape[-2] // 128
    A = A.reshape(A.shape[:-2] + (G, 128, P))
    A = np.swapaxes(A, -1, -2)  # (..., G, P, 128)
    return A.reshape(A.shape[:-3] + (G * 128 * P,))


def _host_prep(x, edge_index):
    src = np.asarray(edge_index[0], dtype=np.int64)
    dst = np.asarray(edge_index[1], dtype=np.int64)
    E = src.shape[0]

    core = dst // ROWS_PER_CORE
    n_loc = dst % ROWS_PER_CORE
    chunk = src // CHUNK
    s_loc = (src % CHUNK).astype(np.int32)

    combo = core * N_CHUNKS + chunk
    gkey = combo * ROWS_PER_CORE + n_loc
    order = np.argsort(gkey, kind="stable")
    gs = gkey[order]
    sl = s_loc[order]

    first = np.empty(E, dtype=bool)
    first[0] = True
    np.not_equal(gs[1:], gs[:-1], out=first[1:])
    gstart = np.flatnonzero(first)
    gid = np.cumsum(first) - 1
    rank = np.arange(E, dtype=np.int64) - gstart[gid]

    deg = np.bincount(gkey, minlength=32 * ROWS_PER_CORE).reshape(32, ROWS_PER_CORE)
    e_combo = gs // ROWS_PER_CORE
    e_node = gs % ROWS_PER_CORE
    e_deg = deg[e_combo, e_node]
    assert int(deg.max()) <= CLASSES[-1][1], int(deg.max())

    # class geometry
    cls_pos, cls_G = [], []
    for lo, hi, P in CLASSES:
        m = (deg >= lo) & (deg <= hi)
        cnt = m.sum(axis=1)
        G = max(1, int(-(-cnt.max() // 128)))
        cls_pos.append(np.cumsum(m, axis=1) - 1)
        cls_G.append(G)

    offs = []
    cur = CHUNK + 1
    for G in cls_G:
        offs.append(cur)
        cur += G * 128
    chunk_region = cur  # rows per chunk region (may exceed 32768; only
    # gather INDICES must stay <= 32767 - checked below)

    # ---- level-1 slot table ----
    in_main = (rank < W - 1) | ((e_deg <= W) & (rank < W))
    A1 = np.full((32, ROWS_PAD, W), PAD_IDX, np.int16)
    m = in_main
    A1[e_combo[m], e_node[m], rank[m]] = sl[m]

    # ---- class slot tables ----
    Ac = []
    for i, (lo, hi, P) in enumerate(CLASSES):
        S = cls_G[i] * 128
        Aci = np.full((32, S, P), PAD_IDX, np.int16)
        m = (~in_main) & (e_deg >= lo) & (e_deg <= hi)
        ec, en = e_combo[m], e_node[m]
        Aci[ec, cls_pos[i][ec, en], rank[m] - (W - 1)] = sl[m]
        Ac.append(Aci)

    # ---- pointers into scratch (level-1 slot W-1) ----
    for i, (lo, hi, P) in enumerate(CLASSES):
        mnode = (deg >= lo) & (deg <= hi)
        ci, ni = np.nonzero(mnode)
        p_ = cls_pos[i][ci, ni]
        G_ = cls_G[i]
        subg = CLS_SUBG[P]
        g_ = p_ // 128
        r_ = p_ % 128
        g0_ = (g_ // subg) * subg
        gsz_ = np.minimum(G_, g0_ + subg) - g0_
        ptr = offs[i] + g0_ * 128 + r_ * gsz_ + (g_ - g0_)
        assert int(ptr.max()) <= 32767, (i, int(ptr.max()))
        A1[ci, ni, W - 1] = ptr

    idx1 = _wrap16(_gather_order(A1, W)).reshape(8, N_CHUNKS, 128, -1)
    cls_idx = [
        _wrap16(_gather_order(Ac[i], CLASSES[i][2])).reshape(8, N_CHUNKS, 128, -1)
        for i in range(len(CLASSES))
    ]

    # ---- x_dev with per-chunk scratch regions ----
    x = np.asarray(x, dtype=np.float32)
    x_dev = np.zeros((N_CHUNKS * chunk_region, D), np.float32)
    for c in range(N_CHUNKS):
        x_dev[c * chunk_region : c * chunk_region + CHUNK] = x[c * CHUNK : (c + 1) * CHUNK]

    return x_dev, idx1, cls_idx, tuple(cls_G), chunk_region


def _build_program(cls_G, chunk_region):
    import concourse.tile as tile
    from concourse import bacc, mybir

    f32 = mybir.dt.float32
    i16 = mybir.dt.int16
    add = mybir.AluOpType.add

    nc = bacc.Bacc(
        "TRN2",
        target_bir_lowering=False,
        debug=False,
        enable_asserts=False,
        num_devices=N_CORES,
        num_swdge_queues=4,
    )
    x_t = nc.dram_tensor("x_dev", [N_CHUNKS * chunk_region, D], f32, kind="ExternalInput")
    idx1_t = [
        nc.dram_tensor(f"idx1_c{c}", [128, N_TILES * TILE_SLOTS // 16], i16, kind="ExternalInput")
        for c in range(N_CHUNKS)
    ]
    cls_t = []
    for i, (lo, hi, P) in enumerate(CLASSES):
        S = cls_G[i] * 128
        cls_t.append(
            [
                nc.dram_tensor(f"idx_cls{i}_c{c}", [128, S * P // 16], i16, kind="ExternalInput")
                for c in range(N_CHUNKS)
            ]
        )
    out_t = nc.dram_tensor("out", [ROWS_PAD, D], f32, kind="ExternalOutput")

    regions = [x_t.ap()[c * chunk_region : (c + 1) * chunk_region] for c in range(N_CHUNKS)]
    out_ap = out_t.ap()

    offs = []
    cur = CHUNK + 1
    for G in cls_G:
        offs.append(cur)
        cur += G * 128

    QCOLS = GROUPS_PER_TILE * W * D     # 1344 staging cols per level-1 tile
    TPD = 1                             # level-1 tiles per gather command
    QCOLS_STG = QCOLS * TPD             # unified staging tile size

    with tile.TileContext(nc) as tc:
        with (
            tc.tile_pool(name="idxr", bufs=1) as idxr_pool,
            tc.tile_pool(name="gstage", bufs=4) as gstage_pool,
            tc.tile_pool(name="ctmp", bufs=2) as ctmp_pool,
            tc.tile_pool(name="cred", bufs=2) as cred_pool,
            tc.tile_pool(name="t1", bufs=2) as t1_pool,
            tc.tile_pool(name="t3", bufs=2) as t3_pool,
            tc.tile_pool(name="outp", bufs=2) as out_pool,
        ):
            # class tables first: phase A needs them immediately; idx1 can
            # stream in behind while phase A runs.
            cls_sb = [None] * len(CLASSES)
            for i in (1, 2, 3, 4, 5, 0):
                S = cls_G[i] * 128
                row = []
                for c in range(N_CHUNKS):
                    t_ = idxr_pool.tile([128, S * cls_t[i][c].shape[1] * 16 // S // 16], i16, tag=f"cls{i}_{c}") if False else idxr_pool.tile([128, S * CLASSES[i][2] // 16], i16, tag=f"cls{i}_{c}")
                    nc.sync.dma_start(t_[:], cls_t[i][c].ap()[:])
                    row.append(t_)
                cls_sb[i] = row
            idx1_sb = []
            for c in range(N_CHUNKS):
                t_ = idxr_pool.tile([128, N_TILES * TILE_SLOTS // 16], i16, tag=f"idx1_{c}")
                nc.sync.dma_start(t_[:], idx1_t[c].ap()[:])
                idx1_sb.append(t_)

            def cls_tree(stg, gsz, P, dst_view, c):
                """Reduce P slot planes of staging tile stg (cols gsz*P*D)
                -> dst_view [128,gsz,64]. Adjacent-pair halving (no step
                slices; AP step slicing is unsupported)."""
                cur, curP, bufi = stg, P, 0
                while True:
                    if curP == 2:
                        v = cur[:, : gsz * 2 * D].rearrange(
                            "p (g k f) -> p g k f", k=2, f=D
                        )
                        nc.vector.tensor_tensor(dst_view, v[:, :, 0, :], v[:, :, 1, :], op=add)
                        return
                    if curP == 3:
                        v = cur[:, : gsz * 3 * D].rearrange(
                            "p (g k f) -> p g k f", k=3, f=D
                        )
                        tt = ctmp_pool.tile([128, gsz * D], f32, tag=f"ct{c}_{bufi}")
                        tv = tt[:, : gsz * D].rearrange("p (g f) -> p g f", f=D)
                        nc.vector.tensor_tensor(tv, v[:, :, 0, :], v[:, :, 1, :], op=add)
                        nc.vector.tensor_tensor(dst_view, tv, v[:, :, 2, :], op=add)
                        return
                    h = curP // 2
                    pv = cur[:, : gsz * curP * D].rearrange(
                        "p (g h two f) -> p g h two f", h=h, two=2, f=D
                    )
                    tt = ctmp_pool.tile([128, gsz * h * D], f32, tag=f"ct{c}_{bufi}")
                    tv = tt[:, : gsz * h * D].rearrange("p (g k f) -> p g k f", k=h, f=D)
                    nc.vector.tensor_tensor(tv, pv[:, :, :, 0, :], pv[:, :, :, 1, :], op=add)
                    cur, curP, bufi = tt, h, bufi + 1

            # ---- phase A: overflow classes -> scratch, queue q = chunk q ----
            # build (class, subblock) work list; interleave chunks inside
            scratch_writes = [[] for _ in range(N_CHUNKS)]
            work = []
            CLS_ORDER = (1, 2, 3, 4, 5, 0)  # end on class 0: its 1-op DVE
            # chain shortens the scratch->level-1 handoff
            for i in CLS_ORDER:
                G = cls_G[i]
                subg = CLS_SUBG[CLASSES[i][2]]
                for g0 in range(0, G, subg):
                    g1 = min(G, g0 + subg)
                    work.append((i, g0, g1))
            for (i, g0, g1) in work:
                lo, hi, P = CLASSES[i]
                gsz = g1 - g0
                nsl = gsz * 128 * P
                for c in range(N_CHUNKS):
                    stg = gstage_pool.tile([128, QCOLS_STG], f32, tag=f"gs{c}")
                    nc.gpsimd.dma_gather(
                        stg[:, : nsl // 128 * D].rearrange("p (s f) -> p s f", f=D),
                        regions[c][: CHUNK + 1],
                        cls_sb[i][c][:, g0 * 128 * P // 16 : g1 * 128 * P // 16],
                        nsl,
                        nsl,
                        D,
                        single_packet=False,
                        queue_num=c,
                    )
                    red = cred_pool.tile([128, gsz * D], f32, tag=f"cr{c}")
                    red_v = red[:, : gsz * D].rearrange("p (g f) -> p g f", f=D)
                    cls_tree(stg, gsz, P, red_v, c)
                    base = offs[i] + g0 * 128
                    dview = regions[c][base : base + gsz * 128].rearrange(
                        "(r g) f -> r (g f)", r=128
                    )
                    winst = nc.sync.dma_start(dview, red[:, : gsz * D])
                    scratch_writes[c].append(winst)

            # ---- phase B: level-1 tiles, per-chunk staging ----
            # Per-chunk staging + per-chunk partial reduces keep each SWDGE
            # queue's buffer recycling independent of the other queues, so
            # their transfer drains can interleave instead of locking step
            # behind one shared staging tile.
            DBL = TPD * GROUPS_PER_TILE  # groups per dispatch (14)
            for t in range(0, N_TILES, TPD):
                parts = []
                for c in range(N_CHUNKS):
                    stg = gstage_pool.tile([128, QCOLS_STG], f32, tag=f"gs{c}")
                    ginst = nc.gpsimd.dma_gather(
                        stg[:].rearrange("p (s f) -> p s f", f=D),
                        regions[c],
                        idx1_sb[c][:, t * IDX_COLS : (t + TPD) * IDX_COLS],
                        TILE_SLOTS * TPD,
                        TILE_SLOTS * TPD,
                        D,
                        single_packet=False,
                        queue_num=c,
                    )
                    if t == 0:
                        # DRAM RAW deps (scratch write -> gather) are not
                        # auto-tracked; pin the first gather per SWDGE
                        # queue, FIFO order covers the rest.
                        for winst in scratch_writes[c]:
                            tile.add_dep_helper(
                                ginst.ins,
                                winst.ins,
                                sync=True,
                                reason="level-1 gather reads overflow scratch",
                            )
                    sv = stg[:].rearrange("p (g k f) -> p g k f", k=W, f=D)
                    tt = t1_pool.tile([128, DBL * D], f32, tag=f"kt{c}")
                    ttv = tt[:].rearrange("p (g f) -> p g f", f=D)
                    nc.vector.tensor_tensor(ttv, sv[:, :, 0, :], sv[:, :, 1, :], op=add)
                    qc = t1_pool.tile([128, DBL * D], f32, tag=f"qp{c}")
                    qcv = qc[:].rearrange("p (g f) -> p g f", f=D)
                    nc.vector.tensor_tensor(qcv, ttv, sv[:, :, 2, :], op=add)
                    parts.append(qcv)
                r0 = t3_pool.tile([128, DBL * D], f32, tag="r0")
                r0v = r0[:].rearrange("p (g f) -> p g f", f=D)
                nc.vector.tensor_tensor(r0v, parts[0], parts[1], op=add)
                r1 = t3_pool.tile([128, DBL * D], f32, tag="r1")
                r1v = r1[:].rearrange("p (g f) -> p g f", f=D)
                nc.vector.tensor_tensor(r1v, parts[2], parts[3], op=add)
                ot = out_pool.tile([128, DBL * D], f32, tag="out")
                otv = ot[:].rearrange("p (g f) -> p g f", f=D)
                nc.vector.tensor_tensor(otv, r0v, r1v, op=add)
                dview = out_ap[t * NODE_TILE : (t + TPD) * NODE_TILE].rearrange(
                    "(r g) f -> r (g f)", r=128
                )
                nc.sync.dma_start(dview, ot[:])

    nc.compile()
    return nc


def kernel(x, edge_index):
    from concourse import bass_utils

    x = np.asarray(x, dtype=np.float32)
    edge_index = np.asarray(edge_index)

    x_dev, idx1, cls_idx, cls_G, chunk_region = _host_prep(x, edge_index)
    sig = (cls_G, chunk_region)
    nc = _PROG_CACHE.get(sig)
    if nc is None:
        nc = _build_program(cls_G, chunk_region)
        _PROG_CACHE[sig] = nc

    in_maps = []
    for core in range(N_CORES):
        m = {"x_dev": x_dev}
        for c in range(N_CHUNKS):
            m[f"idx1_c{c}"] = idx1[core, c]
        for i in range(len(CLASSES)):
            for c in range(N_CHUNKS):
                m[f"idx_cls{i}_c{c}"] = cls_idx[i][core, c]
        in_maps.append(m)

    res = bass_utils.run_bass_kernel_spmd(nc, in_maps, core_ids=list(range(N_CORES)))

    perm = _slab_row(np.arange(ROWS_PER_CORE))
    out = np.empty((N, D), np.float32)
    for core in range(N_CORES):
        slab = res.results[core]["out"]
        out[core * ROWS_PER_CORE : (core + 1) * ROWS_PER_CORE] = slab[perm]
    return out


# revision 39
# speedup vs baseline: 1.2090x; 1.2090x over previous
"""Trainium2 Bass kernel for GNN message passing (gather + segment_sum).

out[i] = sum_{e: dst[e]==i} x[src[e]]   with x [100000, 64] f32,
edge_index [2, 1600000] int64.

Strategy (8 NeuronCores, SPMD), v2:
  - Destination nodes sharded across cores: core c owns dst rows
    [c*12500, (c+1)*12500), padded to a 12544-row output slab whose row
    order makes every device write contiguous (host un-permutes).
  - Source nodes split into 4 chunks of 25000 rows so dma_gather's int16
    indices stay in range. Each chunk region in HBM carries the 25000 x
    rows, a zero pad row, and per-class scratch rows.
  - Per (node, chunk): W=3 level-1 slots. Degree d<=3 fits entirely; for
    d>3 slots 0-1 hold ranks 0-1 and slot 2 is a POINTER to a scratch
    row holding the partial sum of ranks 2.. computed by an overflow
    class gather (classes sized by overflow count: P in {2,3,4,6,8,16}).
    No separate patch phase: the pointer rides the level-1 gather.
  - Device: overflow classes first (gather -> small tree reduce ->
    scratch write), then 14 level-1 tiles, each one 4-chunk mega gather
    into a single staging tile reduced with 4 large strided vector ops.
    SWDGE queue q handles chunk q throughout -> 4-way parallel
    descriptor generation.
"""

import sys

if "/opt/trn_rl_repo" not in sys.path:
    sys.path.insert(0, "/opt/trn_rl_repo")

import numpy as np

N = 100000
D = 64
N_CORES = 8
ROWS_PER_CORE = N // N_CORES            # 12500
NODE_TILE = 896                         # 7 groups of 128 nodes
GROUPS_PER_TILE = NODE_TILE // 128      # 7
N_TILES = 14
ROWS_PAD = NODE_TILE * N_TILES          # 12544
N_CHUNKS = 4
CHUNK = N // N_CHUNKS                   # 25000
PAD_IDX = CHUNK                         # gather index of the zero row
W = 3                                   # level-1 slots per (node, chunk)
TILE_SLOTS = NODE_TILE * W              # 2688 level-1 indices per (tile, chunk)
IDX_COLS = TILE_SLOTS // 16             # 168

# overflow classes: (lo_deg, hi_deg, P slots). Node with deg d>W gets
# ranks 0..W-2 in level 1 + pointer; class holds d-(W-1) ranks, so
# require hi - (W-1) <= P.
CLASSES = ((4, 4, 2), (5, 5, 3), (6, 6, 4), (7, 8, 6), (9, 10, 8), (11, 18, 16))
# subblock group counts: cap staging at ~1536 f32 cols per partition
CLS_SUBG = {2: 8, 3: 5, 4: 4, 6: 2, 8: 2, 16: 1}

_PROG_CACHE = {}


def _wrap16(a):
    """[..., L] int -> [..., 128, L/16] int16 in the dma_gather index layout:
    position i at [i % 16, i // 16], replicated to all queue core pairs."""
    a = np.ascontiguousarray(a.astype(np.int16))
    L = a.shape[-1]
    assert L % 16 == 0
    t = a.reshape(a.shape[:-1] + (L // 16, 16))
    t = np.swapaxes(t, -1, -2)
    reps = (1,) * (a.ndim - 1) + (8, 1)
    return np.ascontiguousarray(np.tile(t, reps))


DISPATCH_TILE = 896  # node-tiles written per level-1 DMA dispatch


def _slab_row(n):
    """Node index within a core -> output slab row (contiguous tile DMAs)."""
    t = n // DISPATCH_TILE
    w = n % DISPATCH_TILE
    g = w // 128
    r = w % 128
    return t * DISPATCH_TILE + r * (DISPATCH_TILE // 128) + g


def _gather_order(A, P):
    """[..., nodes(G*128), P] slots -> flat gather list order (g, k, r)."""
    G = A.shape[-2] // 128
    A = A.reshape(A.shape[:-2] + (G, 128, P))
    A = np.swapaxes(A, -1, -2)  # (..., G, P, 128)
    return A.reshape(A.shape[:-3] + (G * 128 * P,))


def _host_prep(x, edge_index):
    src = np.asarray(edge_index[0], dtype=np.int64)
    dst = np.asarray(edge_index[1], dtype=np.int64)
    E = src.shape[0]

    core = dst // ROWS_PER_CORE
    n_loc = dst % ROWS_PER_CORE
    chunk = src // CHUNK
    s_loc = (src % CHUNK).astype(np.int32)

    combo = core * N_CHUNKS + chunk
    gkey = combo * ROWS_PER_CORE + n_loc
    order = np.argsort(gkey, kind="stable")
    gs = gkey[order]
    sl = s_loc[order]

    first = np.empty(E, dtype=bool)
    first[0] = True
    np.not_equal(gs[1:], gs[:-1], out=first[1:])
    gstart = np.flatnonzero(first)
    gid = np.cumsum(first) - 1
    rank = np.arange(E, dtype=np.int64) - gstart[gid]

    deg = np.bincount(gkey, minlength=32 * ROWS_PER_CORE).reshape(32, ROWS_PER_CORE)
    e_combo = gs // ROWS_PER_CORE
    e_node = gs % ROWS_PER_CORE
    e_deg = deg[e_combo, e_node]
    assert int(deg.max()) <= CLASSES[-1][1], int(deg.max())

    # class geometry
    cls_pos, cls_G = [], []
    for lo, hi, P in CLASSES:
        m = (deg >= lo) & (deg <= hi)
        cnt = m.sum(axis=1)
        G = max(1, int(-(-cnt.max() // 128)))
        cls_pos.append(np.cumsum(m, axis=1) - 1)
        cls_G.append(G)

    offs = []
    cur = CHUNK + 1
    for G in cls_G:
        offs.append(cur)
        cur += G * 128
    chunk_region = cur  # rows per chunk region (may exceed 32768; only
    # gather INDICES must stay <= 32767 - checked below)

    # ---- level-1 slot table ----
    in_main = (rank < W - 1) | ((e_deg <= W) & (rank < W))
    A1 = np.full((32, ROWS_PAD, W), PAD_IDX, np.int16)
    m = in_main
    A1[e_combo[m], e_node[m], rank[m]] = sl[m]

    # ---- class slot tables ----
    Ac = []
    for i, (lo, hi, P) in enumerate(CLASSES):
        S = cls_G[i] * 128
        Aci = np.full((32, S, P), PAD_IDX, np.int16)
        m = (~in_main) & (e_deg >= lo) & (e_deg <= hi)
        ec, en = e_combo[m], e_node[m]
        Aci[ec, cls_pos[i][ec, en], rank[m] - (W - 1)] = sl[m]
        Ac.append(Aci)

    # ---- pointers into scratch (level-1 slot W-1) ----
    for i, (lo, hi, P) in enumerate(CLASSES):
        mnode = (deg >= lo) & (deg <= hi)
        ci, ni = np.nonzero(mnode)
        p_ = cls_pos[i][ci, ni]
        G_ = cls_G[i]
        subg = CLS_SUBG[P]
        g_ = p_ // 128
        r_ = p_ % 128
        g0_ = (g_ // subg) * subg
        gsz_ = np.minimum(G_, g0_ + subg) - g0_
        ptr = offs[i] + g0_ * 128 + r_ * gsz_ + (g_ - g0_)
        assert int(ptr.max()) <= 32767, (i, int(ptr.max()))
        A1[ci, ni, W - 1] = ptr

    idx1 = _wrap16(_gather_order(A1, W)).reshape(8, N_CHUNKS, 128, -1)
    cls_idx = [
        _wrap16(_gather_order(Ac[i], CLASSES[i][2])).reshape(8, N_CHUNKS, 128, -1)
        for i in range(len(CLASSES))
    ]

    # ---- x_dev with per-chunk scratch regions ----
    x = np.asarray(x, dtype=np.float32)
    x_dev = np.zeros((N_CHUNKS * chunk_region, D), np.float32)
    for c in range(N_CHUNKS):
        x_dev[c * chunk_region : c * chunk_region + CHUNK] = x[c * CHUNK : (c + 1) * CHUNK]

    return x_dev, idx1, cls_idx, tuple(cls_G), chunk_region


def _build_program(cls_G, chunk_region):
    import concourse.tile as tile
    from concourse import bacc, mybir

    f32 = mybir.dt.float32
    i16 = mybir.dt.int16
    add = mybir.AluOpType.add

    nc = bacc.Bacc(
        "TRN2",
        target_bir_lowering=False,
        debug=False,
        enable_asserts=False,
        num_devices=N_CORES,
        num_swdge_queues=4,
    )
    x_t = nc.dram_tensor("x_dev", [N_CHUNKS * chunk_region, D], f32, kind="ExternalInput")
    idx1_t = [
        nc.dram_tensor(f"idx1_c{c}", [128, N_TILES * TILE_SLOTS // 16], i16, kind="ExternalInput")
        for c in range(N_CHUNKS)
    ]
    cls_t = []
    for i, (lo, hi, P) in enumerate(CLASSES):
        S = cls_G[i] * 128
        cls_t.append(
            [
                nc.dram_tensor(f"idx_cls{i}_c{c}", [128, S * P // 16], i16, kind="ExternalInput")
                for c in range(N_CHUNKS)
            ]
        )
    out_t = nc.dram_tensor("out", [ROWS_PAD, D], f32, kind="ExternalOutput")

    regions = [x_t.ap()[c * chunk_region : (c + 1) * chunk_region] for c in range(N_CHUNKS)]
    out_ap = out_t.ap()

    offs = []
    cur = CHUNK + 1
    for G in cls_G:
        offs.append(cur)
        cur += G * 128

    QCOLS = GROUPS_PER_TILE * W * D     # 1344 staging cols per level-1 tile
    TPD = 1                             # level-1 tiles per gather command
    QCOLS_STG = QCOLS * TPD             # unified staging tile size

    with tile.TileContext(nc) as tc:
        with (
            tc.tile_pool(name="idxr", bufs=1) as idxr_pool,
            tc.tile_pool(name="gstage", bufs=4) as gstage_pool,
            tc.tile_pool(name="ctmp", bufs=2) as ctmp_pool,
            tc.tile_pool(name="cred", bufs=3) as cred_pool,
            tc.tile_pool(name="idxp", bufs=3) as idxp_pool,
            tc.tile_pool(name="t1", bufs=2) as t1_pool,
            tc.tile_pool(name="t3", bufs=2) as t3_pool,
            tc.tile_pool(name="outp", bufs=2) as out_pool,
        ):
            # class tables first: phase A needs them immediately; idx1 can
            # stream in behind while phase A runs.
            cls_sb = [None] * len(CLASSES)
            for i in (1, 2, 3, 4, 5, 0):
                S = cls_G[i] * 128
                row = []
                for c in range(N_CHUNKS):
                    t_ = idxr_pool.tile([128, S * cls_t[i][c].shape[1] * 16 // S // 16], i16, tag=f"cls{i}_{c}") if False else idxr_pool.tile([128, S * CLASSES[i][2] // 16], i16, tag=f"cls{i}_{c}")
                    nc.sync.dma_start(t_[:], cls_t[i][c].ap()[:])
                    row.append(t_)
                cls_sb[i] = row


            def cls_tree(stg, gsz, P, dst_view, c):
                """Reduce P slot planes of staging tile stg (cols gsz*P*D)
                -> dst_view [128,gsz,64]. Adjacent-pair halving (no step
                slices; AP step slicing is unsupported)."""
                cur, curP, bufi = stg, P, 0
                while True:
                    if curP == 2:
                        v = cur[:, : gsz * 2 * D].rearrange(
                            "p (g k f) -> p g k f", k=2, f=D
                        )
                        nc.vector.tensor_tensor(dst_view, v[:, :, 0, :], v[:, :, 1, :], op=add)
                        return
                    if curP == 3:
                        v = cur[:, : gsz * 3 * D].rearrange(
                            "p (g k f) -> p g k f", k=3, f=D
                        )
                        tt = ctmp_pool.tile([128, gsz * D], f32, tag=f"ct{c}_{bufi}")
                        tv = tt[:, : gsz * D].rearrange("p (g f) -> p g f", f=D)
                        nc.vector.tensor_tensor(tv, v[:, :, 0, :], v[:, :, 1, :], op=add)
                        nc.vector.tensor_tensor(dst_view, tv, v[:, :, 2, :], op=add)
                        return
                    h = curP // 2
                    pv = cur[:, : gsz * curP * D].rearrange(
                        "p (g h two f) -> p g h two f", h=h, two=2, f=D
                    )
                    tt = ctmp_pool.tile([128, gsz * h * D], f32, tag=f"ct{c}_{bufi}")
                    tv = tt[:, : gsz * h * D].rearrange("p (g k f) -> p g k f", k=h, f=D)
                    nc.vector.tensor_tensor(tv, pv[:, :, :, 0, :], pv[:, :, :, 1, :], op=add)
                    cur, curP, bufi = tt, h, bufi + 1

            # ---- phase A: overflow classes -> scratch, queue q = chunk q ----
            # build (class, subblock) work list; interleave chunks inside
            scratch_writes = [[] for _ in range(N_CHUNKS)]
            work = []
            CLS_ORDER = (1, 2, 3, 4, 5, 0)  # end on class 0: its 1-op DVE
            # chain shortens the scratch->level-1 handoff
            for i in CLS_ORDER:
                G = cls_G[i]
                subg = CLS_SUBG[CLASSES[i][2]]
                for g0 in range(0, G, subg):
                    g1 = min(G, g0 + subg)
                    work.append((i, g0, g1))
            for (i, g0, g1) in work:
                lo, hi, P = CLASSES[i]
                gsz = g1 - g0
                nsl = gsz * 128 * P
                for c in range(N_CHUNKS):
                    stg = gstage_pool.tile([128, QCOLS_STG], f32, tag=f"gs{c}")
                    nc.gpsimd.dma_gather(
                        stg[:, : nsl // 128 * D].rearrange("p (s f) -> p s f", f=D),
                        regions[c][: CHUNK + 1],
                        cls_sb[i][c][:, g0 * 128 * P // 16 : g1 * 128 * P // 16],
                        nsl,
                        nsl,
                        D,
                        single_packet=False,
                        queue_num=c,
                    )
                    red = cred_pool.tile([128, gsz * D], f32, tag=f"cr{c}")
                    red_v = red[:, : gsz * D].rearrange("p (g f) -> p g f", f=D)
                    cls_tree(stg, gsz, P, red_v, c)
                    base = offs[i] + g0 * 128
                    dview = regions[c][base : base + gsz * 128].rearrange(
                        "(r g) f -> r (g f)", r=128
                    )
                    winst = nc.sync.dma_start(dview, red[:, : gsz * D])
                    scratch_writes[c].append(winst)

            # ---- phase B: level-1 tiles, per-chunk staging ----
            # Per-chunk staging + per-chunk partial reduces keep each SWDGE
            # queue's buffer recycling independent of the other queues, so
            # their transfer drains can interleave instead of locking step
            # behind one shared staging tile.
            DBL = TPD * GROUPS_PER_TILE  # groups per dispatch (14)
            for t in range(0, N_TILES, TPD):
                parts = []
                for c in range(N_CHUNKS):
                    idxt = idxp_pool.tile([128, TPD * IDX_COLS], i16, tag=f"ix{c}")
                    nc.sync.dma_start(
                        idxt[:], idx1_t[c].ap()[:, t * IDX_COLS : (t + TPD) * IDX_COLS]
                    )
                    stg = gstage_pool.tile([128, QCOLS_STG], f32, tag=f"gs{c}")
                    ginst = nc.gpsimd.dma_gather(
                        stg[:].rearrange("p (s f) -> p s f", f=D),
                        regions[c],
                        idxt[:],
                        TILE_SLOTS * TPD,
                        TILE_SLOTS * TPD,
                        D,
                        single_packet=False,
                        queue_num=c,
                    )
                    if t == 0:
                        # DRAM RAW deps (scratch write -> gather) are not
                        # auto-tracked; pin the first gather per SWDGE
                        # queue, FIFO order covers the rest.
                        for winst in scratch_writes[c]:
                            tile.add_dep_helper(
                                ginst.ins,
                                winst.ins,
                                sync=True,
                                reason="level-1 gather reads overflow scratch",
                            )
                    sv = stg[:].rearrange("p (g k f) -> p g k f", k=W, f=D)
                    tt = t1_pool.tile([128, DBL * D], f32, tag=f"kt{c}")
                    ttv = tt[:].rearrange("p (g f) -> p g f", f=D)
                    nc.vector.tensor_tensor(ttv, sv[:, :, 0, :], sv[:, :, 1, :], op=add)
                    qc = t1_pool.tile([128, DBL * D], f32, tag=f"qp{c}")
                    qcv = qc[:].rearrange("p (g f) -> p g f", f=D)
                    nc.vector.tensor_tensor(qcv, ttv, sv[:, :, 2, :], op=add)
                    parts.append(qcv)
                r0 = t3_pool.tile([128, DBL * D], f32, tag="r0")
                r0v = r0[:].rearrange("p (g f) -> p g f", f=D)
                nc.vector.tensor_tensor(r0v, parts[0], parts[1], op=add)
                r1 = t3_pool.tile([128, DBL * D], f32, tag="r1")
                r1v = r1[:].rearrange("p (g f) -> p g f", f=D)
                nc.vector.tensor_tensor(r1v, parts[2], parts[3], op=add)
                ot = out_pool.tile([128, DBL * D], f32, tag="out")
                otv = ot[:].rearrange("p (g f) -> p g f", f=D)
                nc.vector.tensor_tensor(otv, r0v, r1v, op=add)
                dview = out_ap[t * NODE_TILE : (t + TPD) * NODE_TILE].rearrange(
                    "(r g) f -> r (g f)", r=128
                )
                nc.sync.dma_start(dview, ot[:])

    nc.compile()
    return nc


def kernel(x, edge_index):
    from concourse import bass_utils

    x = np.asarray(x, dtype=np.float32)
    edge_index = np.asarray(edge_index)

    x_dev, idx1, cls_idx, cls_G, chunk_region = _host_prep(x, edge_index)
    sig = (cls_G, chunk_region)
    nc = _PROG_CACHE.get(sig)
    if nc is None:
        nc = _build_program(cls_G, chunk_region)
        _PROG_CACHE[sig] = nc

    in_maps = []
    for core in range(N_CORES):
        m = {"x_dev": x_dev}
        for c in range(N_CHUNKS):
            m[f"idx1_c{c}"] = idx1[core, c]
        for i in range(len(CLASSES)):
            for c in range(N_CHUNKS):
                m[f"idx_cls{i}_c{c}"] = cls_idx[i][core, c]
        in_maps.append(m)

    res = bass_utils.run_bass_kernel_spmd(nc, in_maps, core_ids=list(range(N_CORES)))

    perm = _slab_row(np.arange(ROWS_PER_CORE))
    out = np.empty((N, D), np.float32)
    for core in range(N_CORES):
        slab = res.results[core]["out"]
        out[core * ROWS_PER_CORE : (core + 1) * ROWS_PER_CORE] = slab[perm]
    return out


# revision 40
# speedup vs baseline: 1.2136x; 1.0038x over previous
"""Trainium2 Bass kernel for GNN message passing (gather + segment_sum).

out[i] = sum_{e: dst[e]==i} x[src[e]]   with x [100000, 64] f32,
edge_index [2, 1600000] int64.

Strategy (8 NeuronCores, SPMD), v2:
  - Destination nodes sharded across cores: core c owns dst rows
    [c*12500, (c+1)*12500), padded to a 12544-row output slab whose row
    order makes every device write contiguous (host un-permutes).
  - Source nodes split into 4 chunks of 25000 rows so dma_gather's int16
    indices stay in range. Each chunk region in HBM carries the 25000 x
    rows, a zero pad row, and per-class scratch rows.
  - Per (node, chunk): W=3 level-1 slots. Degree d<=3 fits entirely; for
    d>3 slots 0-1 hold ranks 0-1 and slot 2 is a POINTER to a scratch
    row holding the partial sum of ranks 2.. computed by an overflow
    class gather (classes sized by overflow count: P in {2,3,4,6,8,16}).
    No separate patch phase: the pointer rides the level-1 gather.
  - Device: overflow classes first (gather -> small tree reduce ->
    scratch write), then 14 level-1 tiles, each one 4-chunk mega gather
    into a single staging tile reduced with 4 large strided vector ops.
    SWDGE queue q handles chunk q throughout -> 4-way parallel
    descriptor generation.
"""

import sys

if "/opt/trn_rl_repo" not in sys.path:
    sys.path.insert(0, "/opt/trn_rl_repo")

import numpy as np

N = 100000
D = 64
N_CORES = 8
ROWS_PER_CORE = N // N_CORES            # 12500
NODE_TILE = 896                         # 7 groups of 128 nodes
GROUPS_PER_TILE = NODE_TILE // 128      # 7
N_TILES = 14
ROWS_PAD = NODE_TILE * N_TILES          # 12544
N_CHUNKS = 4
CHUNK = N // N_CHUNKS                   # 25000
PAD_IDX = CHUNK                         # gather index of the zero row
W = 3                                   # level-1 slots per (node, chunk)
TILE_SLOTS = NODE_TILE * W              # 2688 level-1 indices per (tile, chunk)
IDX_COLS = TILE_SLOTS // 16             # 168

# overflow classes: (lo_deg, hi_deg, P slots). Node with deg d>W gets
# ranks 0..W-2 in level 1 + pointer; class holds d-(W-1) ranks, so
# require hi - (W-1) <= P.
CLASSES = ((4, 4, 2), (5, 5, 3), (6, 6, 4), (7, 8, 6), (9, 10, 8), (11, 18, 16))
# subblock group counts: cap staging at ~1536 f32 cols per partition
CLS_SUBG = {2: 8, 3: 5, 4: 4, 6: 2, 8: 2, 16: 1}

_PROG_CACHE = {}


def _wrap16(a):
    """[..., L] int -> [..., 128, L/16] int16 in the dma_gather index layout:
    position i at [i % 16, i // 16], replicated to all queue core pairs."""
    a = np.ascontiguousarray(a.astype(np.int16))
    L = a.shape[-1]
    assert L % 16 == 0
    t = a.reshape(a.shape[:-1] + (L // 16, 16))
    t = np.swapaxes(t, -1, -2)
    reps = (1,) * (a.ndim - 1) + (8, 1)
    return np.ascontiguousarray(np.tile(t, reps))


DISPATCH_TILE = 896  # node-tiles written per level-1 DMA dispatch


def _slab_row(n):
    """Node index within a core -> output slab row (contiguous tile DMAs)."""
    t = n // DISPATCH_TILE
    w = n % DISPATCH_TILE
    g = w // 128
    r = w % 128
    return t * DISPATCH_TILE + r * (DISPATCH_TILE // 128) + g


def _gather_order(A, P):
    """[..., nodes(G*128), P] slots -> flat gather list order (g, k, r)."""
    G = A.shape[-2] // 128
    A = A.reshape(A.shape[:-2] + (G, 128, P))
    A = np.swapaxes(A, -1, -2)  # (..., G, P, 128)
    return A.reshape(A.shape[:-3] + (G * 128 * P,))


def _host_prep(x, edge_index):
    src = np.asarray(edge_index[0], dtype=np.int64)
    dst = np.asarray(edge_index[1], dtype=np.int64)
    E = src.shape[0]

    core = dst // ROWS_PER_CORE
    n_loc = dst % ROWS_PER_CORE
    chunk = src // CHUNK
    s_loc = (src % CHUNK).astype(np.int32)

    combo = core * N_CHUNKS + chunk
    gkey = combo * ROWS_PER_CORE + n_loc
    order = np.argsort(gkey, kind="stable")
    gs = gkey[order]
    sl = s_loc[order]

    first = np.empty(E, dtype=bool)
    first[0] = True
    np.not_equal(gs[1:], gs[:-1], out=first[1:])
    gstart = np.flatnonzero(first)
    gid = np.cumsum(first) - 1
    rank = np.arange(E, dtype=np.int64) - gstart[gid]

    deg = np.bincount(gkey, minlength=32 * ROWS_PER_CORE).reshape(32, ROWS_PER_CORE)
    e_combo = gs // ROWS_PER_CORE
    e_node = gs % ROWS_PER_CORE
    e_deg = deg[e_combo, e_node]
    assert int(deg.max()) <= CLASSES[-1][1], int(deg.max())

    # class geometry
    cls_pos, cls_G = [], []
    for lo, hi, P in CLASSES:
        m = (deg >= lo) & (deg <= hi)
        cnt = m.sum(axis=1)
        G = max(1, int(-(-cnt.max() // 128)))
        cls_pos.append(np.cumsum(m, axis=1) - 1)
        cls_G.append(G)

    offs = []
    cur = CHUNK + 1
    for G in cls_G:
        offs.append(cur)
        cur += G * 128
    chunk_region = cur  # rows per chunk region (may exceed 32768; only
    # gather INDICES must stay <= 32767 - checked below)

    # ---- level-1 slot table ----
    in_main = (rank < W - 1) | ((e_deg <= W) & (rank < W))
    A1 = np.full((32, ROWS_PAD, W), PAD_IDX, np.int16)
    m = in_main
    A1[e_combo[m], e_node[m], rank[m]] = sl[m]

    # ---- class slot tables ----
    Ac = []
    for i, (lo, hi, P) in enumerate(CLASSES):
        S = cls_G[i] * 128
        Aci = np.full((32, S, P), PAD_IDX, np.int16)
        m = (~in_main) & (e_deg >= lo) & (e_deg <= hi)
        ec, en = e_combo[m], e_node[m]
        Aci[ec, cls_pos[i][ec, en], rank[m] - (W - 1)] = sl[m]
        Ac.append(Aci)

    # ---- pointers into scratch (level-1 slot W-1) ----
    for i, (lo, hi, P) in enumerate(CLASSES):
        mnode = (deg >= lo) & (deg <= hi)
        ci, ni = np.nonzero(mnode)
        p_ = cls_pos[i][ci, ni]
        G_ = cls_G[i]
        subg = CLS_SUBG[P]
        g_ = p_ // 128
        r_ = p_ % 128
        g0_ = (g_ // subg) * subg
        gsz_ = np.minimum(G_, g0_ + subg) - g0_
        ptr = offs[i] + g0_ * 128 + r_ * gsz_ + (g_ - g0_)
        assert int(ptr.max()) <= 32767, (i, int(ptr.max()))
        A1[ci, ni, W - 1] = ptr

    idx1 = _wrap16(_gather_order(A1, W)).reshape(8, N_CHUNKS, 128, -1)
    cls_idx = [
        _wrap16(_gather_order(Ac[i], CLASSES[i][2])).reshape(8, N_CHUNKS, 128, -1)
        for i in range(len(CLASSES))
    ]

    # ---- x_dev with per-chunk scratch regions ----
    x = np.asarray(x, dtype=np.float32)
    x_dev = np.zeros((N_CHUNKS * chunk_region, D), np.float32)
    for c in range(N_CHUNKS):
        x_dev[c * chunk_region : c * chunk_region + CHUNK] = x[c * CHUNK : (c + 1) * CHUNK]

    return x_dev, idx1, cls_idx, tuple(cls_G), chunk_region


def _build_program(cls_G, chunk_region):
    import concourse.tile as tile
    from concourse import bacc, mybir

    f32 = mybir.dt.float32
    i16 = mybir.dt.int16
    add = mybir.AluOpType.add

    nc = bacc.Bacc(
        "TRN2",
        target_bir_lowering=False,
        debug=False,
        enable_asserts=False,
        num_devices=N_CORES,
        num_swdge_queues=4,
    )
    x_t = nc.dram_tensor("x_dev", [N_CHUNKS * chunk_region, D], f32, kind="ExternalInput")
    idx1_t = [
        nc.dram_tensor(f"idx1_c{c}", [128, N_TILES * TILE_SLOTS // 16], i16, kind="ExternalInput")
        for c in range(N_CHUNKS)
    ]
    cls_t = []
    for i, (lo, hi, P) in enumerate(CLASSES):
        S = cls_G[i] * 128
        cls_t.append(
            [
                nc.dram_tensor(f"idx_cls{i}_c{c}", [128, S * P // 16], i16, kind="ExternalInput")
                for c in range(N_CHUNKS)
            ]
        )
    out_t = nc.dram_tensor("out", [ROWS_PAD, D], f32, kind="ExternalOutput")

    regions = [x_t.ap()[c * chunk_region : (c + 1) * chunk_region] for c in range(N_CHUNKS)]
    out_ap = out_t.ap()

    offs = []
    cur = CHUNK + 1
    for G in cls_G:
        offs.append(cur)
        cur += G * 128

    QCOLS = GROUPS_PER_TILE * W * D     # 1344 staging cols per level-1 tile
    TPD = 1                             # level-1 tiles per gather command
    QCOLS_STG = QCOLS * TPD             # unified staging tile size

    with tile.TileContext(nc) as tc:
        with (
            tc.tile_pool(name="idxr", bufs=1) as idxr_pool,
            tc.tile_pool(name="gstage", bufs=4) as gstage_pool,
            tc.tile_pool(name="ctmp", bufs=2) as ctmp_pool,
            tc.tile_pool(name="cred", bufs=3) as cred_pool,
            tc.tile_pool(name="idxp", bufs=3) as idxp_pool,
            tc.tile_pool(name="t1", bufs=2) as t1_pool,
            tc.tile_pool(name="t3", bufs=2) as t3_pool,
            tc.tile_pool(name="outp", bufs=2) as out_pool,
        ):
            # class tables first: phase A needs them immediately; idx1 can
            # stream in behind while phase A runs.
            cls_sb = [None] * len(CLASSES)
            for i in (1, 2, 3, 4, 5, 0):
                S = cls_G[i] * 128
                row = []
                for c in range(N_CHUNKS):
                    t_ = idxr_pool.tile([128, S * cls_t[i][c].shape[1] * 16 // S // 16], i16, tag=f"cls{i}_{c}") if False else idxr_pool.tile([128, S * CLASSES[i][2] // 16], i16, tag=f"cls{i}_{c}")
                    nc.sync.dma_start(t_[:], cls_t[i][c].ap()[:])
                    row.append(t_)
                cls_sb[i] = row


            def cls_tree(stg, gsz, P, dst_view, c):
                """Reduce P slot planes of staging tile stg (cols gsz*P*D)
                -> dst_view [128,gsz,64]. Adjacent-pair halving (no step
                slices; AP step slicing is unsupported)."""
                cur, curP, bufi = stg, P, 0
                while True:
                    if curP == 2:
                        v = cur[:, : gsz * 2 * D].rearrange(
                            "p (g k f) -> p g k f", k=2, f=D
                        )
                        nc.vector.tensor_tensor(dst_view, v[:, :, 0, :], v[:, :, 1, :], op=add)
                        return
                    if curP == 3:
                        v = cur[:, : gsz * 3 * D].rearrange(
                            "p (g k f) -> p g k f", k=3, f=D
                        )
                        tt = ctmp_pool.tile([128, gsz * D], f32, tag=f"ct{c}_{bufi}")
                        tv = tt[:, : gsz * D].rearrange("p (g f) -> p g f", f=D)
                        nc.vector.tensor_tensor(tv, v[:, :, 0, :], v[:, :, 1, :], op=add)
                        nc.vector.tensor_tensor(dst_view, tv, v[:, :, 2, :], op=add)
                        return
                    h = curP // 2
                    pv = cur[:, : gsz * curP * D].rearrange(
                        "p (g h two f) -> p g h two f", h=h, two=2, f=D
                    )
                    tt = ctmp_pool.tile([128, gsz * h * D], f32, tag=f"ct{c}_{bufi}")
                    tv = tt[:, : gsz * h * D].rearrange("p (g k f) -> p g k f", k=h, f=D)
                    nc.vector.tensor_tensor(tv, pv[:, :, :, 0, :], pv[:, :, :, 1, :], op=add)
                    cur, curP, bufi = tt, h, bufi + 1

            # ---- phase A: overflow classes -> scratch, queue q = chunk q ----
            # build (class, subblock) work list; interleave chunks inside
            scratch_writes = [[] for _ in range(N_CHUNKS)]
            work = []
            CLS_ORDER = (1, 2, 3, 4, 5, 0)  # end on class 0: its 1-op DVE
            # chain shortens the scratch->level-1 handoff
            for i in CLS_ORDER:
                G = cls_G[i]
                subg = CLS_SUBG[CLASSES[i][2]]
                for g0 in range(0, G, subg):
                    g1 = min(G, g0 + subg)
                    work.append((i, g0, g1))
            for (i, g0, g1) in work:
                lo, hi, P = CLASSES[i]
                gsz = g1 - g0
                nsl = gsz * 128 * P
                for c in range(N_CHUNKS):
                    stg = gstage_pool.tile([128, QCOLS_STG], f32, tag=f"gs{c}")
                    nc.gpsimd.dma_gather(
                        stg[:, : nsl // 128 * D].rearrange("p (s f) -> p s f", f=D),
                        regions[c][: CHUNK + 1],
                        cls_sb[i][c][:, g0 * 128 * P // 16 : g1 * 128 * P // 16],
                        nsl,
                        nsl,
                        D,
                        single_packet=False,
                        queue_num=c,
                    )
                    red = cred_pool.tile([128, gsz * D], f32, tag=f"cr{c}")
                    red_v = red[:, : gsz * D].rearrange("p (g f) -> p g f", f=D)
                    cls_tree(stg, gsz, P, red_v, c)
                    base = offs[i] + g0 * 128
                    dview = regions[c][base : base + gsz * 128].rearrange(
                        "(r g) f -> r (g f)", r=128
                    )
                    winst = nc.sync.dma_start(dview, red[:, : gsz * D])
                    scratch_writes[c].append(winst)

            # ---- phase B: level-1 tiles, per-chunk staging ----
            # Per-chunk staging + per-chunk partial reduces keep each SWDGE
            # queue's buffer recycling independent of the other queues, so
            # their transfer drains can interleave instead of locking step
            # behind one shared staging tile.
            DBL = TPD * GROUPS_PER_TILE  # groups per dispatch (14)
            PF = 2  # idx-load prefetch depth (tiles) past the current one

            def load_idx(t):
                row = []
                for c in range(N_CHUNKS):
                    idxt = idxp_pool.tile([128, TPD * IDX_COLS], i16, tag=f"ix{c}")
                    nc.sync.dma_start(
                        idxt[:], idx1_t[c].ap()[:, t * IDX_COLS : (t + TPD) * IDX_COLS]
                    )
                    row.append(idxt)
                return row

            tiles_seq = list(range(0, N_TILES, TPD))
            idx_tiles = {}
            for t in tiles_seq[: PF + 1]:
                idx_tiles[t] = load_idx(t)
            for ti, t in enumerate(tiles_seq):
                if ti + PF + 1 < len(tiles_seq):
                    tpre = tiles_seq[ti + PF + 1]
                    idx_tiles[tpre] = load_idx(tpre)
                parts = []
                for c in range(N_CHUNKS):
                    stg = gstage_pool.tile([128, QCOLS_STG], f32, tag=f"gs{c}")
                    ginst = nc.gpsimd.dma_gather(
                        stg[:].rearrange("p (s f) -> p s f", f=D),
                        regions[c],
                        idx_tiles[t][c][:],
                        TILE_SLOTS * TPD,
                        TILE_SLOTS * TPD,
                        D,
                        single_packet=False,
                        queue_num=c,
                    )
                    if t == 0:
                        # DRAM RAW deps (scratch write -> gather) are not
                        # auto-tracked; pin the first gather per SWDGE
                        # queue, FIFO order covers the rest.
                        for winst in scratch_writes[c]:
                            tile.add_dep_helper(
                                ginst.ins,
                                winst.ins,
                                sync=True,
                                reason="level-1 gather reads overflow scratch",
                            )
                    sv = stg[:].rearrange("p (g k f) -> p g k f", k=W, f=D)
                    tt = t1_pool.tile([128, DBL * D], f32, tag=f"kt{c}")
                    ttv = tt[:].rearrange("p (g f) -> p g f", f=D)
                    nc.vector.tensor_tensor(ttv, sv[:, :, 0, :], sv[:, :, 1, :], op=add)
                    qc = t1_pool.tile([128, DBL * D], f32, tag=f"qp{c}")
                    qcv = qc[:].rearrange("p (g f) -> p g f", f=D)
                    nc.vector.tensor_tensor(qcv, ttv, sv[:, :, 2, :], op=add)
                    parts.append(qcv)
                r0 = t3_pool.tile([128, DBL * D], f32, tag="r0")
                r0v = r0[:].rearrange("p (g f) -> p g f", f=D)
                nc.vector.tensor_tensor(r0v, parts[0], parts[1], op=add)
                r1 = t3_pool.tile([128, DBL * D], f32, tag="r1")
                r1v = r1[:].rearrange("p (g f) -> p g f", f=D)
                nc.vector.tensor_tensor(r1v, parts[2], parts[3], op=add)
                ot = out_pool.tile([128, DBL * D], f32, tag="out")
                otv = ot[:].rearrange("p (g f) -> p g f", f=D)
                nc.vector.tensor_tensor(otv, r0v, r1v, op=add)
                dview = out_ap[t * NODE_TILE : (t + TPD) * NODE_TILE].rearrange(
                    "(r g) f -> r (g f)", r=128
                )
                nc.sync.dma_start(dview, ot[:])

    nc.compile()
    return nc


def kernel(x, edge_index):
    from concourse import bass_utils

    x = np.asarray(x, dtype=np.float32)
    edge_index = np.asarray(edge_index)

    x_dev, idx1, cls_idx, cls_G, chunk_region = _host_prep(x, edge_index)
    sig = (cls_G, chunk_region)
    nc = _PROG_CACHE.get(sig)
    if nc is None:
        nc = _build_program(cls_G, chunk_region)
        _PROG_CACHE[sig] = nc

    in_maps = []
    for core in range(N_CORES):
        m = {"x_dev": x_dev}
        for c in range(N_CHUNKS):
            m[f"idx1_c{c}"] = idx1[core, c]
        for i in range(len(CLASSES)):
            for c in range(N_CHUNKS):
                m[f"idx_cls{i}_c{c}"] = cls_idx[i][core, c]
        in_maps.append(m)

    res = bass_utils.run_bass_kernel_spmd(nc, in_maps, core_ids=list(range(N_CORES)))

    perm = _slab_row(np.arange(ROWS_PER_CORE))
    out = np.empty((N, D), np.float32)
    for core in range(N_CORES):
        slab = res.results[core]["out"]
        out[core * ROWS_PER_CORE : (core + 1) * ROWS_PER_CORE] = slab[perm]
    return out
